# revision 10
# baseline (speedup 1.0000x reference)
"""Trainium2 Bass kernel for CenterDependentPool2D (v3).

Input  x: (8, 64, 448, 448) fp32  ->  Output: (8, 64, 224, 224) fp32.

Per core = one batch element.  Partition p = c + 64*wg: channel c, wg 0 =
out cols 0..111 (natural j), wg 1 = out cols 223..112 (MIRRORED local j).
The mirror is applied by the Activation-engine fp32->fp16 casts (strided /
reversed reads are free there), so every DVE op is a unified 128-partition
instruction and each ring occupies a single low-j column interval =>
per-band column gating of the whole pyramid.

Five ring windows (k in {2,8,14,20,26}, stride 2, reflect pad == clip)
decompose over pair arrays E[i]=max(x[2i],x[2i+1]), O[i]=max(x[2i+1],
x[2i+2]) in both dims.  32-row out bands (amortize the ~0.5us DVE drain
per instruction): Act casts de-interleaved column-parity arrays, DVE
builds pair maxes + shifted-max doubling pyramids (fp16 tensor_tensor,
2x mode), ring combines and blend (nested-disk copy_predicated) are
column-gated to ring bounding boxes; output stored fp16, upcast on host.
"""

import numpy as np

import concourse.bass as bass
import concourse.mybir as mybir
from concourse.tile import TileContext
from concourse.bass_utils import run_bass_kernel_spmd

# ---------------- problem constants ----------------
B, C, IN, OUT = 8, 64, 448, 224
CEN = 112
OW = 112
EW = 124          # pair-array width
WIN = 250         # input chunk cols (incl pads)
NEG = -30000.0
RADII = (60, 75, 90, 105)
DT = mybir.dt.float16
MX = mybir.AluOpType.max

# out-row bands: [0,24), [24,56), ..., [184,216), [216,224)
BANDS = [(0, 24)] + [(24 + 32 * k, 56 + 32 * k) for k in range(6)] \
    + [(216, 224)]
NBANDS = len(BANDS)

# ---------------- static geometry ----------------

_yy, _xx = np.mgrid[0:OUT, 0:OUT]
_D2 = (_yy - CEN) ** 2 + (_xx - CEN) ** 2
NESTED = np.stack([(_D2 < R * R) for R in RADII])
RING_ID = 4 - NESTED.sum(0)


def _localize(a):
    return a[:, 0:CEN], a[:, ::-1][:, 0:CEN]


def _hull(a, b):
    if a is None:
        return b
    if b is None:
        return a
    return (min(a[0], b[0]), max(a[1], b[1]))


class BandGeom:
    def __init__(self, it):
        self.it = it
        y0, y1 = BANDS[it]
        self.y0, self.y1, self.H = y0, y1, y1 - y0
        r0, r1 = _localize(RING_ID)
        rows = slice(y0, y1)
        self.ring = []
        for r in range(5):
            m = (r0[rows] == r) | (r1[rows] == r)
            if not m.any():
                self.ring.append(None)
                continue
            ridx = np.where(m.any(1))[0]
            cidx = np.where(m.any(0))[0]
            self.ring.append(dict(
                rlo=y0 + int(ridx.min()), rhi=y0 + int(ridx.max()) + 1,
                clo=int(cidx.min()), chi=int(cidx.max()) + 1))
        assert self.ring[4] is not None and self.ring[4]["clo"] == 0
        assert self.ring[4]["rlo"] == y0 and self.ring[4]["rhi"] == y1
        self.b4 = self.ring[4]["chi"]
        # ring0 inscribed square: cols where every ring0-bbox row is inside
        # disk60 for both wg variants (unconditional copy, no mask needed)
        self.sq0 = None
        if self.ring[0] is not None:
            g0 = self.ring[0]
            n0, n1 = _localize(NESTED[0])
            rs = slice(g0["rlo"], g0["rhi"])
            allin = n0[rs].all(0) & n1[rs].all(0)
            ci = np.where(allin)[0]
            if len(ci) and ci.max() - ci.min() >= 8:
                self.sq0 = (int(ci.min()), int(ci.max()) + 1)
                assert self.sq0[0] >= g0["clo"] and self.sq0[1] <= g0["chi"]

    def blend_mask(self, r):
        g = self.ring[r]
        n0, n1 = _localize(NESTED[r])
        s0 = n0[g["rlo"]:g["rhi"], g["clo"]:g["chi"]].astype(np.uint8)
        s1 = n1[g["rlo"]:g["rhi"], g["clo"]:g["chi"]].astype(np.uint8)
        m = np.zeros((128,) + s0.shape, np.uint8)
        m[0:64] = s0[None]
        m[64:128] = s1[None]
        return m


class Extents:
    """Backward-propagated (rows, cols) per pyramid level; rows in E/O-row
    (== out-row) space, cols in pair-e space, half-open."""

    def __init__(self, g):
        y0, y1 = g.y0, g.y1
        s13_r, s13_c = (y0 - 6, y1 - 6), (0, g.b4)
        v13_r, v13_c = s13_r, (s13_c[0], s13_c[1] + 5)
        s8_r, s8_c = (v13_r[0], v13_r[1] + 5), v13_c
        a8_r, a8_c = s8_r, (s8_c[0], s8_c[1] + 4)
        r2 = g.ring[2]
        if r2 is not None:
            s7_r = (r2["rlo"] - 3, r2["rhi"] - 3)
            s7_c = (r2["clo"] + 3, r2["chi"] + 3)
            u7_r, u7_c = s7_r, (s7_c[0], s7_c[1] + 3)
            s4_r = _hull((a8_r[0], a8_r[1] + 4), (u7_r[0], u7_r[1] + 3))
            s4_c = _hull(a8_c, u7_c)
        else:
            s7_r = s7_c = u7_r = u7_c = None
            s4_r, s4_c = (a8_r[0], a8_r[1] + 4), a8_c
        a4_r, a4_c = s4_r, (s4_c[0], s4_c[1] + 2)
        s2_r, s2_c = (a4_r[0], a4_r[1] + 2), a4_c
        a2_r, a2_c = s2_r, (s2_c[0], s2_c[1] + 1)
        self.ee_rows = (a2_r[0], a2_r[1] + 1)

        r1, r3 = g.ring[1], g.ring[3]
        s10_r = s10_c = w10_r = w10_c = None
        s8o_r = s8o_c = a8o_r = a8o_c = None
        s4o_r = s4o_c = None
        if r1 is not None:
            s4o_r = (r1["rlo"] - 2, r1["rhi"] - 2)
            s4o_c = (r1["clo"] + 4, r1["chi"] + 4)
        if r3 is not None:
            s10_r = (r3["rlo"] - 5, r3["rhi"] - 5)
            s10_c = (r3["clo"] + 1, r3["chi"] + 1)
            w10_r, w10_c = s10_r, (s10_c[0], s10_c[1] + 2)
            s8o_r, s8o_c = (w10_r[0], w10_r[1] + 2), w10_c
            a8o_r, a8o_c = s8o_r, (s8o_c[0], s8o_c[1] + 4)
            s4o_r = _hull(s4o_r, (a8o_r[0], a8o_r[1] + 4))
            s4o_c = _hull(s4o_c, a8o_c)
        if s4o_r is not None:
            a4o_r, a4o_c = s4o_r, (s4o_c[0], s4o_c[1] + 2)
            s2o_r, s2o_c = (a4o_r[0], a4o_r[1] + 2), a4o_c
            a2o_r, a2o_c = s2o_r, (s2o_c[0], s2o_c[1] + 1)
            self.oo_rows = (a2o_r[0], a2o_r[1] + 1)
            self.oo_cols = (a2o_c[0], a2o_c[1] + 1)
        else:
            a4o_r = a4o_c = s2o_r = s2o_c = a2o_r = a2o_c = None
            self.oo_rows = None
            self.oo_cols = None

        self.lv = dict(
            a2=(a2_r, a2_c), s2=(s2_r, s2_c), a4=(a4_r, a4_c),
            s4=(s4_r, s4_c), a8=(a8_r, a8_c), s8=(s8_r, s8_c),
            u7=(u7_r, u7_c), s7=(s7_r, s7_c), v13=(v13_r, v13_c),
            s13=(s13_r, s13_c),
            a2o=(a2o_r, a2o_c), s2o=(s2o_r, s2o_c), a4o=(a4o_r, a4o_c),
            s4o=(s4o_r, s4o_c), a8o=(a8o_r, a8o_c), s8o=(s8o_r, s8o_c),
            w10=(w10_r, w10_c), s10=(s10_r, s10_c),
        )


GEOMS = [BandGeom(it) for it in range(NBANDS)]
EXTENTS = [Extents(g) for g in GEOMS]

# columns the O-side arrays must carry per band: this band's pyramid needs
# union next band's (carry rows serve it)
OWG = []
for _it in range(NBANDS):
    _a = EXTENTS[_it].oo_cols
    _b = EXTENTS[_it + 1].oo_cols if _it + 1 < NBANDS else None
    _u = _hull(_a, _b)
    OWG.append(_u if _u is not None else (0, 124))

for _g, _e in zip(GEOMS, EXTENTS):
    _ob = 32 * _g.it - 14
    assert _e.ee_rows[0] >= _ob and _e.ee_rows[1] <= _ob + 46
    if _e.oo_rows is not None:
        assert _e.oo_rows[0] >= _ob and _e.oo_rows[1] <= _ob + 46
    for _n, (_rr, _cc) in _e.lv.items():
        if _cc is not None:
            assert 0 <= _cc[0] and _cc[1] <= 125, (_g.it, _n, _cc)

# pooled level tiles: tag sharing by disjoint lifetime
LV_TAG = dict(a2="tP", a4="tP", a8="tP", v13="tP",
              s2="tQ", s8="tQ", s4="tS4", u7="tT", w10="tT",
              s7="tS7", a2o="tPo", a4o="tPo", a8o="tPo",
              s2o="tQo", s8o="tQo", s4o="tS4o", s10="tS10")
TAG_MAX = {}
for _e in EXTENTS:
    for _n, (_rr, _cc) in _e.lv.items():
        if _n == "s13" or _rr is None:
            continue
        t = LV_TAG[_n]
        sz = TAG_MAX.get(t, (0, 0))
        TAG_MAX[t] = (max(sz[0], _rr[1] - _rr[0]),
                      max(sz[1], _cc[1] - _cc[0]))


def _build_strips():
    blobs, bands, offs = [], [], []
    pos = 0
    for g in GEOMS:
        start = pos
        ent = {}
        for r in (3, 2, 1, 0):
            if g.ring[r] is None:
                continue
            m = g.blend_mask(r)
            nr, nc = m.shape[1], m.shape[2]
            if r == 0 and g.sq0 is not None:
                clo = g.ring[0]["clo"]
                ja, jb = g.sq0
                for key, mm in (("0L", m[:, :, 0:ja - clo]),
                                ("0R", m[:, :, jb - clo:])):
                    if mm.shape[2] == 0:
                        continue
                    ent[key] = (pos - start, nr, mm.shape[2])
                    blobs.append(np.ascontiguousarray(mm).reshape(128, -1))
                    pos += nr * mm.shape[2]
                continue
            ent[r] = (pos - start, nr, nc)
            blobs.append(m.reshape(128, -1))
            pos += nr * nc
        offs.append(ent)
        bands.append((start, pos - start))
    blob = (np.concatenate(blobs, 1) if blobs
            else np.zeros((128, 1), np.uint8))
    return blob, bands, offs


STRIP_BLOB, STRIP_BANDS, STRIP_OFFS = _build_strips()
STRIP_MAX = max(sz for _, sz in STRIP_BANDS)

BLEND_SRC = {3: ("s10", -5, 1), 2: ("s7", -3, 3),
             1: ("s4o", -2, 4), 0: ("ee", 0, 6)}

# ---------------- shared band program ----------------


def _emit_band(be, it):
    g, E = GEOMS[it], EXTENTS[it]
    y0, y1, H, b4 = g.y0, g.y1, g.H, g.b4
    OB = 32 * it - 14
    ee, oo, Ew, Ow, ol = be.ee, be.oo, be.Ew, be.Ow, be.owlast

    glo, ghi = OWG[it]
    if it < 7:
        az = be.azeo()
        for c in range(2):
            be.dma_in(it, c, az)
            r0, r1 = 32 * c, 32 * c + 32
            be.max2(Ew[:, r0:r1, :], az[:, r0:r1, 0:124],
                    az[:, r0:r1, 126:250])
            be.max2(Ow[:, r0:r1, glo:ghi], az[:, r0:r1, 126 + glo:126 + ghi],
                    az[:, r0:r1, 1 + glo:1 + ghi])

    if it == 0:
        be.memset(ee[:, 0:14, :], NEG)
        be.memset(oo[:, 0:14, :], NEG)
    else:
        be.gcopy(ee[:, 0:14, :], ee[:, 32:46, :])
        be.gcopy(oo[:, 0:14, :], oo[:, 32:46, :])

    # straddler row 13 = O[32*it - 1]
    slo, shi = EXTENTS[it].oo_cols if EXTENTS[it].oo_cols else (0, 1)
    if it == 0:
        be.scopy(oo[:, 13:14, slo:shi], Ow[:, 0:1, slo:shi])
    elif it == 7:
        be.scopy(oo[:, 13:14, slo:shi], ol[:, 0:1, slo:shi])
    else:
        be.max2(oo[:, 13:14, slo:shi], ol[:, 0:1, slo:shi],
                Ow[:, 0:1, slo:shi])

    if it < 7:
        be.max2(ee[:, 14:46, :], Ew[:, 0:64:2, :], Ew[:, 1:64:2, :])
        be.max2(oo[:, 14:45, glo:ghi],
                Ow[:, 1:63:2, glo:ghi], Ow[:, 2:64:2, glo:ghi])
        be.acopy(ol[:, 0:1, glo:ghi], Ow[:, 63:64, glo:ghi])
    else:
        be.memset(ee[:, 14:46, :], NEG)
        be.memset(oo[:, 14:46, :], NEG)

    # ---- pyramids ----
    P = {"ee": (ee, OB, 0), "oo": (oo, OB, 0)}
    out16 = be.out16()

    def comb(name, src, d, axis, dst=None):
        rr, cc = E.lv[name]
        nr, nc = rr[1] - rr[0], cc[1] - cc[0]
        st, sr0, sc0 = P[src]
        ra, rb = rr[0] - sr0, rr[1] - sr0
        ca, cb = cc[0] - sc0, cc[1] - sc0
        assert ra >= 0 and ca >= 0, (it, name)
        if axis == "r":
            a = st[:, ra:rb, ca:cb]
            b = st[:, ra + d:rb + d, ca:cb]
        else:
            a = st[:, ra:rb, ca:cb]
            b = st[:, ra:rb, ca + d:cb + d]
        if dst is None:
            t = be.lv(name)
            be.max2(t[:, 0:nr, 0:nc], a, b)
            P[name] = (t, rr[0], cc[0])
        else:
            be.max2(dst, a, b)
            P[name] = None

    comb("a2", "ee", 1, "r")
    comb("s2", "a2", 1, "c")
    comb("a4", "s2", 2, "r")
    comb("s4", "a4", 2, "c")
    comb("a8", "s4", 4, "r")
    comb("s8", "a8", 4, "c")
    comb("v13", "s8", 5, "r")
    comb("s13", "v13", 5, "c", dst=out16[:, 0:H, 0:b4])
    if g.ring[2] is not None:
        comb("u7", "s4", 3, "r")
        comb("s7", "u7", 3, "c")
    if E.oo_rows is not None:
        comb("a2o", "oo", 1, "r")
        comb("s2o", "a2o", 1, "c")
        comb("a4o", "s2o", 2, "r")
        comb("s4o", "a4o", 2, "c")
        if g.ring[3] is not None:
            comb("a8o", "s4o", 4, "r")
            comb("s8o", "a8o", 4, "c")
            comb("w10", "s8o", 2, "r")
            comb("s10", "w10", 2, "c")

    # ---- blend ----
    if any(g.ring[r] is not None for r in (3, 2, 1, 0)):
        be.dma_strip(it)
    for r in (3, 2, 1, 0):
        if g.ring[r] is None:
            continue
        gg = g.ring[r]
        rlo, rhi = gg["rlo"], gg["rhi"]
        sname, roff, coff = BLEND_SRC[r]
        st, sr0, sc0 = P[sname]
        ra, rb = rlo + roff - sr0, rhi + roff - sr0

        def seg(clo, chi, key, masked):
            if chi <= clo:
                return
            ca, cb = clo + coff - sc0, chi + coff - sc0
            assert ra >= 0 and ca >= 0, (it, r)
            data = st[:, ra:rb, ca:cb]
            dst = out16[:, rlo - y0:rhi - y0, clo:chi]
            if masked:
                be.cp(dst, be.strip_ap(it, key), data)
            else:
                be.scopy(dst, data)

        if r == 0 and g.sq0 is not None:
            ja, jb = g.sq0
            seg(gg["clo"], ja, "0L", True)
            seg(ja, jb, None, False)
            seg(jb, gg["chi"], "0R", True)
        else:
            seg(gg["clo"], gg["chi"], r, True)

    be.dma_out(it, out16)


def _emit_program(be):
    for it in range(NBANDS):
        _emit_band(be, it)


def prep_input(x1):
    """x1 [C, 448, 448] fp32 -> [2, C, 448, 250] fp16 parity-split blob.
    Per (wg, c, row): [zE (125) | pad | zO (124)], wg1 mirrored, NEG pads.
    Pure layout marshalling (cast/reorder/pad) -- no arithmetic."""
    xz = np.full((2, C, IN, 250), NEG, np.float16)
    xz[0, :, :, 6:125] = x1[:, :, 0:237:2]       # zE0[e]=x[2e-12]
    xz[0, :, :, 132:250] = x1[:, :, 1:236:2]     # zO0[e]=x[2e-11]
    xz[1, :, :, 6:125] = x1[:, :, 447:209:-2]    # zE1[e]=x[459-2e]
    xz[1, :, :, 132:250] = x1[:, :, 446:210:-2]  # zO1[e]=x[458-2e]
    return xz


# ---------------- numpy backend (validation) ----------------


class NumpyBE:
    def __init__(self, x):
        self.xz = prep_input(x).astype(np.float32)
        f32 = np.float32
        self._azeo = np.full((128, 64, 250), np.nan, f32)
        self.Ew = np.full((128, 64, 124), np.nan, f32)
        self.Ow = np.full((128, 64, 124), np.nan, f32)
        self.owlast = np.full((128, 1, 124), np.nan, f32)
        self.ee = np.full((128, 46, 124), np.nan, f32)
        self.oo = np.full((128, 46, 124), np.nan, f32)
        self.y = np.full((C, OUT, OUT), np.nan, f32)
        self._chunk = None
        self._out = None
        self._flip = None

    def azeo(self):
        return self._azeo

    def lv(self, name):
        nr, nc = TAG_MAX[LV_TAG[name]]
        return np.full((128, nr, nc), np.nan, np.float32)

    def out16(self):
        self._out = np.full((128, 32, OW), np.nan, np.float32)
        return self._out

    def outflip(self):
        self._flip = np.full((128, 32, OW), np.nan, np.float32)
        return self._flip

    def memset(self, ap, v):
        ap[...] = v

    def max2(self, d, a, b):
        assert d.shape == a.shape == b.shape, (d.shape, a.shape, b.shape)
        np.maximum(a, b, out=d)

    def scopy(self, d, s):
        d[...] = s

    acopy = scopy
    gcopy = scopy
    cast = scopy

    def cp(self, out, mask, data):
        assert out.shape == mask.shape == data.shape
        out[...] = np.where(mask != 0, data, out)

    def dma_in(self, it, c, az):
        r0 = 64 * it + 32 * c
        az[0:64, 32 * c:32 * c + 32, :] = self.xz[0, :, r0:r0 + 32, :]
        az[64:128, 32 * c:32 * c + 32, :] = self.xz[1, :, r0:r0 + 32, :]

    def dma_strip(self, it):
        pass

    def strip_ap(self, it, r):
        start, _ = STRIP_BANDS[it]
        off, nr, nc = STRIP_OFFS[it][r]
        return STRIP_BLOB[:, start + off:start + off + nr * nc].reshape(
            128, nr, nc)

    def dma_out(self, it, out16):
        g = GEOMS[it]
        self.y[:, g.y0:g.y1, 0:OW] = out16[0:64, 0:g.H, :]
        self.y[:, g.y0:g.y1, OW:OUT] = out16[64:128, 0:g.H, ::-1]


def numpy_kernel(x1):
    """x1: [64, 448, 448] -> [64, 224, 224] (fp32, exact clip semantics)."""
    be = NumpyBE(x1)
    _emit_program(be)
    assert not np.isnan(be.y).any(), "uncovered output pixels"
    return be.y


# ---------------- bass backend ----------------


def split_multi_waits(nc):
    """walrus CoreV3Gen accepts at most 1 sync-wait per instruction; Tile's
    tail drains can carry 2+.  Peel extras onto preceding NoOps."""
    n = 0
    for fn in nc.m.functions:
        for bb in fn.blocks:
            insts = list(bb.instructions)
            out = []
            for ins in insts:
                si = getattr(ins, "sync_info", None)
                if si is not None and len(si.on_wait) > 1:
                    waits = list(si.on_wait)
                    for k, w in enumerate(waits[:-1]):
                        nop = mybir.InstNoOp(
                            name=f"{ins.name}-waitsplit{k}",
                            engine=ins.engine, ins=[], outs=[])
                        nop.sync_info = mybir.SyncInfo(
                            on_wait=[w], on_update=[])
                        out.append(nop)
                        n += 1
                    ins.sync_info = mybir.SyncInfo(
                        on_wait=[waits[-1]], on_update=list(si.on_update))
                out.append(ins)
            if n:
                bb.instructions = out
    return n


class BassBE:
    def __init__(self, nc, pools, x, y, strips):
        self.nc = nc
        self.x = x
        self.y = y
        self.strips = strips
        pers, self.lvpool, self.iop, self.chpool = pools
        f32 = mybir.dt.float32
        self.Ew = pers.tile([128, 64, 124], DT, tag="Ew")
        self.Ow = pers.tile([128, 64, 124], DT, tag="Ow")
        self.owlast = pers.tile([128, 1, 124], DT, tag="owlast")
        self.ee = pers.tile([128, 46, 124], DT, tag="ee")
        self.oo = pers.tile([128, 46, 124], DT, tag="oo")
        self._f32 = f32
        self._strip = None

    def azeo(self):
        return self.chpool.tile([128, 64, 250], DT, tag="azeo", name="azeo")

    def lv(self, name):
        nr, nc_ = TAG_MAX[LV_TAG[name]]
        return self.lvpool.tile([128, nr, nc_], DT, tag=LV_TAG[name], name=f"lv_{name}")

    def out16(self):
        return self.iop.tile([128, 32, OW], DT, tag="out16", name="out16")

    def outflip(self):
        return self.iop.tile([128, 32, OW], DT, tag="oflip", name="oflip")

    def memset(self, ap, v):
        self.nc.gpsimd.memset(ap, v)

    def max2(self, d, a, b):
        self.nc.vector.tensor_tensor(d, a, b, MX)

    def scopy(self, d, s):
        self.nc.vector.tensor_scalar_max(d, s, NEG)

    def acopy(self, d, s):
        self.nc.scalar.copy(d, s)

    gcopy = acopy

    def cast(self, d, s):
        self.nc.scalar.copy(d, s)

    def cp(self, out, mask, data):
        self.nc.vector.copy_predicated(out, mask, data)

    def dma_in(self, it, c, az):
        r0 = 64 * it + 32 * c
        self.nc.sync.dma_start(az[0:64, 32 * c:32 * c + 32, :],
                               self.x[0, :, r0:r0 + 32, :])
        self.nc.sync.dma_start(az[64:128, 32 * c:32 * c + 32, :],
                               self.x[1, :, r0:r0 + 32, :])

    def dma_strip(self, it):
        start, sz = STRIP_BANDS[it]
        self._strip = self.iop.tile([128, STRIP_MAX], mybir.dt.uint8,
                                    tag="strip", name="strip")
        self.nc.sync.dma_start(self._strip[:, 0:sz],
                               self.strips[:, start:start + sz])

    def strip_ap(self, it, r):
        off, nr, nc_ = STRIP_OFFS[it][r]
        return self._strip[:, off:off + nr * nc_].rearrange(
            "p (r c) -> p r c", c=nc_)

    def dma_out(self, it, out16):
        g = GEOMS[it]
        self.nc.sync.dma_start(self.y[0, :, g.y0:g.y1, :],
                               out16[0:64, 0:g.H, :])
        self.nc.sync.dma_start(self.y[1, :, g.y0:g.y1, :],
                               out16[64:128, 0:g.H, :])


def _emit_kernel(nc: bass.Bass):
    x = nc.dram_tensor("x", [2, C, IN, 250], DT,
                       kind="ExternalInput")
    y = nc.dram_tensor("y", [2, C, OUT, OW], DT,
                       kind="ExternalOutput")
    strips = nc.inline_tensor(STRIP_BLOB, name="mstrips")

    with TileContext(nc) as tc:
        with tc.tile_pool(name="pp", bufs=1) as pers, \
             tc.tile_pool(name="lv", bufs=1) as lvpool, \
             tc.tile_pool(name="io", bufs=1) as iop, \
             tc.tile_pool(name="ch", bufs=2) as chpool:
            be = BassBE(nc, (pers, lvpool, iop, chpool), x, y, strips)
            _emit_program(be)
    return nc


_CACHED = {}


def _get_nc():
    if "nc" not in _CACHED:
        nc = bass.Bass()
        _emit_kernel(nc)
        split_multi_waits(nc)
        _CACHED["nc"] = nc
    return _CACHED["nc"]


def kernel(x: np.ndarray) -> np.ndarray:
    nc = _get_nc()
    in_maps = [{"x": prep_input(x[b].astype(np.float32))}
               for b in range(B)]
    res = run_bass_kernel_spmd(nc, in_maps, core_ids=list(range(B)))
    out = np.empty((B, C, OUT, OUT), np.float32)
    for b, r in enumerate(res.results):
        yw = r["y"].astype(np.float32)      # [2, C, 224, 112]
        out[b, :, :, 0:OW] = yw[0]
        out[b, :, :, OW:OUT] = yw[1][:, :, ::-1]
    return out
